# revision 9
# baseline (speedup 1.0000x reference)
"""Batched signature kernel (Goursat PDE) on 8 NeuronCores.

Math: per pair, K_diff = diff2(x @ y.T) = dx @ dy.T where dx/dy are path
increments.  DYADIC_ORDER=1 doubles the grid: A[i,j] = K_diff[i//2, j//2]/4 - 1
on a 510x510 grid.  PDE u[i+1,j+1] = u[i+1,j] + u[i,j+1] + u[i,j]*A[i,j] is,
per row, a first-order recurrence -> one DVE tensor_tensor_scan per row:
    state = (u_prev[j+1] + state) + tmp[j],  tmp = u_prev[j]*A[i,j]
Sharding: batch 256 pairs -> 32 per core, pairs on SBUF partitions.
All inputs are packed into ONE DRAM tensor / ONE DMA (walrus only allows a
single embedded sync-wait on PE matmul instructions).
"""
import functools
import sys

import numpy as np

sys.path.insert(0, "/opt/trn_rl_repo")

import concourse.bass as bass
import concourse.bacc as bacc
import concourse.mybir as mybir
from concourse import tile
from concourse.bass_utils import run_bass_kernel_spmd

B, L, D = 256, 256, 64
NCORES = 8
BP = B // NCORES        # 32 pairs per core
LM = L - 1              # 255 increments
N2 = 2 * LM             # 510 PDE grid size
NBLK = 17               # A-row streaming blocks
BLK = LM // NBLK        # 15 A rows per block
F32 = mybir.dt.float32
ADD = mybir.AluOpType.add
COPY = mybir.ActivationFunctionType.Copy

XSZ = BP * 2 * D        # 4096 cols of packed x per partition
PACK = 2 * XSZ + 2 * LM  # + 510 cols of deltaT


def _build_program():
    nc = bacc.Bacc(None, target_bir_lowering=False)
    pk_d = nc.declare_dram_parameter("packed", [128, PACK], F32, isOutput=False)
    out_d = nc.declare_dram_parameter("out", [BP, 1], F32, isOutput=True)
    A_d = nc.dram_tensor("A_scratch", [BP, LM, LM], F32)

    with tile.TileContext(nc) as tc:
        with (
            tc.tile_pool(name="const", bufs=1) as cpool,
            tc.tile_pool(name="ps", bufs=2, space="PSUM") as pspool,
            tc.tile_pool(name="ev", bufs=3) as evpool,
            tc.tile_pool(name="pde", bufs=1) as upool,
            tc.tile_pool(name="ablk", bufs=2) as apool,
            tc.tile_pool(name="tmp", bufs=2) as tpool,
        ):
            pk = cpool.tile([128, PACK], F32)
            nc.gpsimd.dma_start(out=pk[:], in_=pk_d[:])

            def x_ap(p, c):
                o = p * 2 * D + c * D
                return pk[:, o : o + D]

            def y_ap(p, c):
                o = XSZ + p * 2 * D + c * D
                return pk[:, o : o + D]

            def dT_ap(c):
                o = 2 * XSZ + c * LM
                return pk[:, o : o + LM]

            # ---- preprocessing: A[p] = 0.25 * dx @ dy.T - 1 -> DRAM ----
            for p in range(BP):
                # dxT[d, a] = sum_l x[l, d] * deltaT[l, a]  (contraction over l)
                dxT_ps = pspool.tile([D, LM], F32, tag="dxps", name="dxT_ps")
                dyT_ps = pspool.tile([D, LM], F32, tag="dyps", name="dyT_ps")
                for c in range(2):
                    nc.tensor.matmul(
                        dxT_ps[:], x_ap(p, c), dT_ap(c),
                        start=(c == 0), stop=(c == 1),
                    )
                for c in range(2):
                    nc.tensor.matmul(
                        dyT_ps[:], y_ap(p, c), dT_ap(c),
                        start=(c == 0), stop=(c == 1),
                    )
                dxT_sb = evpool.tile([D, LM], F32, tag="dxe", name="dxT_sb")
                dyT_sb = evpool.tile([D, LM], F32, tag="dye", name="dyT_sb")
                # fold /4 into the factors: (0.5 dx) @ (0.5 dy).T
                nc.scalar.activation(dxT_sb[:], dxT_ps[:], COPY, scale=0.5)
                nc.scalar.activation(dyT_sb[:], dyT_ps[:], COPY, scale=0.5)
                for m0, m1 in ((0, 128), (128, LM)):
                    a_ps = pspool.tile([128, LM], F32, tag="aps", name="a_ps")
                    nc.tensor.matmul(
                        a_ps[: m1 - m0, :], dxT_sb[:, m0:m1], dyT_sb[:],
                        start=True, stop=True,
                    )
                    a_sb = evpool.tile([128, LM], F32, tag="aev", name="a_sb", bufs=64)
                    nc.scalar.activation(
                        a_sb[: m1 - m0, :], a_ps[: m1 - m0, :], COPY, bias=-1.0
                    )
                    nc.sync.dma_start(out=A_d[p][m0:m1, :], in_=a_sb[: m1 - m0, :])

            # ---- PDE: 510 rows, each = elementwise mult + scan ----
            u_bufs = [
                upool.tile([BP, N2 + 1], F32, tag=f"u{i}", name=f"u{i}")
                for i in range(2)
            ]
            nc.vector.memset(u_bufs[0][:], 1.0)
            nc.vector.memset(u_bufs[1][:], 1.0)
            step = 0
            for b in range(NBLK):
                ablk = apool.tile([BP, BLK * LM], F32, tag="ablk", name="ablk")
                nc.sync.dma_start(
                    out=ablk[:],
                    in_=A_d[:, b * BLK : (b + 1) * BLK, :].rearrange(
                        "p r a -> p (r a)"
                    ),
                )
                for r in range(BLK):
                    base = ablk[:, r * LM : (r + 1) * LM]
                    # doubled read: A[a] repeated 2x along free dim (step-0 AP)
                    dbl = bass.AP(
                        base.tensor,
                        base.offset,
                        [base.ap[0], [base.ap[1][0], LM], [0, 2]],
                    )
                    for _ in range(2):
                        up = u_bufs[step % 2]
                        un = u_bufs[(step + 1) % 2]
                        tmp = tpool.tile([BP, N2], F32, tag="tmp", name="tmp")
                        nc.gpsimd.tensor_mul(tmp[:], up[:, 0:N2], dbl)
                        nc.vector.tensor_tensor_scan(
                            un[:, 1 : N2 + 1], up[:, 1 : N2 + 1], tmp[:],
                            1.0, ADD, ADD,
                        )
                        step += 1
            nc.sync.dma_start(out=out_d[:], in_=u_bufs[step % 2][:, N2 : N2 + 1])
    nc.compile()
    return nc


@functools.lru_cache(maxsize=1)
def _program():
    return _build_program()


def _delta_T() -> np.ndarray:
    dT = np.zeros((L, LM), np.float32)
    for a in range(LM):
        dT[a + 1, a] = 1.0
        dT[a, a] = -1.0
    return dT


def _pack(xs_c: np.ndarray, ys_c: np.ndarray, dT: np.ndarray) -> np.ndarray:
    # per-partition packing: partition q holds, for each pair p and chunk c,
    # x[p, c*128+q, :], then same for y, then deltaT rows c*128+q
    xq = xs_c.reshape(BP, 2, 128, D).transpose(2, 0, 1, 3).reshape(128, XSZ)
    yq = ys_c.reshape(BP, 2, 128, D).transpose(2, 0, 1, 3).reshape(128, XSZ)
    dq = dT.reshape(2, 128, LM).transpose(1, 0, 2).reshape(128, 2 * LM)
    return np.ascontiguousarray(np.concatenate([xq, yq, dq], axis=1))


def kernel(xs: np.ndarray, ys: np.ndarray) -> np.ndarray:
    xs = np.asarray(xs, np.float32)
    ys = np.asarray(ys, np.float32)
    dT = _delta_T()
    in_maps = [
        {"packed": _pack(xs[c * BP : (c + 1) * BP], ys[c * BP : (c + 1) * BP], dT)}
        for c in range(NCORES)
    ]
    nc = _program()
    res = run_bass_kernel_spmd(nc, in_maps, list(range(NCORES)))
    return np.concatenate([r["out"][:, 0] for r in res.results])
